# revision 17
# baseline (speedup 1.0000x reference)
"""Trainium2 Bass kernel for nn_CategoryBranch2 (3 conv blocks + 2 BiGRU layers).

Distribution:
  phase 1/2 (convs + gx1) data-parallel: one sample per core (8 cores).
  GRU scans regrouped so each core scans TWO SAME-DIRECTION chains (the two
  samples of its core-pair): even cores scan forward chains, odd cores scan
  backward chains.  The two chains share the recurrent weight matrix, so each
  scan step is 48 matmuls of [128cont x 128out x 2cols] instead of 96 1-col
  matmuls -- half the PE instruction stream, the real bottleneck.

  Regrouping uses pairwise AllGather collectives (groups [0,1],[2,3],...) on
  internal DRAM bounce buffers; each core picks its two chains out of the
  4 gathered slots with one dynamic-offset DMA (slot index is a per-core
  uint32 input 'dsel' = core%2; chains sit at slots {dsel, dsel+2}).

  Pipeline: convs -> gx1 -> AG1(gx1 by dir) -> L1 scans -> AG2(louts by
  sample) -> l2in -> gx2 -> AG3(gx2 by dir) -> L2 scans -> AG4(partials by
  sample) -> sum -> out.  All collectives are pair-local.

Device plan per core (phases 1-2 identical to the data-parallel baseline):
  phase1  conv blocks, T-tiled (8 tiles of 128 steps, halo recompute).
  phase2  gx1[dir] = wi1' @ y' + (bi + bh_rz), fp32 SBUF, bwd stored
          time-reversed; converted to bf16 for the collective.

Self-contained: hardcodes all shapes; host does only numpy weight re-layouts.
"""

import numpy as np
import ml_dtypes

import concourse.bacc as bacc
import concourse.bass as bass
import concourse.mybir as mybir
from concourse.tile import TileContext
from concourse.bass import ds
from concourse.bass_utils import run_bass_kernel_spmd

BF16 = ml_dtypes.bfloat16
F32 = mybir.dt.float32
BF = mybir.dt.bfloat16
U32 = mybir.dt.uint32
AF = mybir.ActivationFunctionType
OP = mybir.AluOpType
PE = mybir.EngineType.PE

BN_EPS = 1e-5
NEG_STRIDE_GX = True   # write bwd gx time-reversed (negative-stride DVE store)

X9_LEN = 134 * 130          # 17420
X2_LEN = 132 * 66 + 66      # 8778
X3_LEN = 130 * 34 + 34      # 4454
YPST_LEN = 32 * 128         # 4096

PAIR_GROUPS = [[0, 1], [2, 3], [4, 5], [6, 7]]

_CACHED_NC = {}


def _pre_triple(nc, ps, ident, bhnl, e8, gxs4, t):
    """Allocate the psum triple for step t and preload it via PE matmuls:
    psr/psz <- gx_rz(t) (identity-stationary copy), psn <- bhn (one-hot rhs).
    Emitted in the PREVIOUS superstep's matmul stream, so the DVE queue stays
    clean for the q->npre->b->f critical chain."""
    psr = ps.tile([128, 8], F32, tag="psr", name="psr")
    psz = ps.tile([128, 8], F32, tag="psz", name="psz")
    psn = ps.tile([128, 8], F32, tag="psn", name="psn")
    nc.tensor.matmul(psn, bhnl, e8, start=True, stop=False,
                     skip_group_check=True)
    nc.tensor.matmul(psr, ident, gxs4[:, 0:4, :, ds(t, 1)],
                     start=True, stop=False, skip_group_check=True)
    nc.tensor.matmul(psz, ident, gxs4[:, 4:8, :, ds(t, 1)],
                     start=True, stop=False, skip_group_check=True)
    return psr, psz, psn


_LAST_TRI = [None]


def _superstep(nc, vs, wh, gxn4, tri, h_old, h_new, louts, t, s_out, pre=None):
    """One time step for TWO same-direction chains (matmul columns).

    Per-gate psum tiles (r/z/n, [128, 8], col = j*2 + ch), preloaded by
    _pre_triple.  Matmul order n -> r -> z so psn's PE->DVE semaphore and the
    r-sigmoid both resolve as early as possible.  All tail ops flat [128, 8].
    h_* [128, 8] bf16: col = kc*2 + ch.
    """
    psr, psz, psn = tri
    for g, pst in ((2, psn), (0, psr), (1, psz)):
        for j in range(4):
            mc = g * 4 + j
            for kc in range(4):
                nc.tensor.matmul(
                    pst[:, j * 2:j * 2 + 2],
                    wh[:, (kc * 12 + mc) * 128:(kc * 12 + mc + 1) * 128],
                    h_old[:, kc * 2:kc * 2 + 2],
                    start=False, stop=(kc == 3), skip_group_check=True)
    if pre is not None:
        _LAST_TRI[0] = pre()
    # tail: sig_r -> q -> npre -> tanh_n -> b -> f -> tanh_h is the serial
    # chain; the z path (sig_z, zc, a) runs beside it on ACT/Pool.
    r = vs.tile([128, 8], BF, tag="r")
    nc.scalar.activation(r, psr, AF.Sigmoid)
    z = vs.tile([128, 8], BF, tag="z")
    nc.scalar.activation(z, psz, AF.Sigmoid)
    q = vs.tile([128, 8], BF, tag="q")
    nc.vector.tensor_tensor(q, r, psn, OP.mult)
    npre = vs.tile([128, 8], BF, tag="npre")
    nc.vector.tensor_tensor(npre[:].rearrange("p (j c) -> p j c", c=2)[:, :, :, None],
                            q[:].rearrange("p (j c) -> p j c", c=2)[:, :, :, None],
                            gxn4[:, :, :, ds(t, 1)], OP.add)
    zc = vs.tile([128, 8], BF, tag="zc")
    nc.gpsimd.tensor_scalar(zc, z, -1.0, 1.0, OP.mult, OP.add)
    a = vs.tile([128, 8], BF, tag="a")
    nc.gpsimd.tensor_mul(a, z, h_old[:])
    n = vs.tile([128, 8], BF, tag="n")
    nc.scalar.activation(n, npre, AF.Tanh)
    b = vs.tile([128, 8], BF, tag="b")
    nc.vector.tensor_tensor(b, zc, n, OP.mult)
    f = vs.tile([128, 8], BF, tag="f")
    nc.vector.tensor_tensor(f, a, b, OP.add)
    nc.scalar.activation(h_new[:], f, AF.Tanh)
    if s_out is not None:
        nc.gpsimd.tensor_copy(
            louts[:, :, :, ds(s_out, 1)],
            h_new[:].rearrange("p (k c) -> p k c", c=2)[:, :, :, None])


def _scan_loop(nc, tc, vs, ps, nblk, wh, gxs4, gxn4, ident, bhnl, e8,
               ha, hb, louts, BLK=64):
    with tc.For_i(0, nblk, 1, hint_engines=(PE,)) as blk:
        base = blk * BLK
        sbase = blk * (BLK // 2)
        tri = _pre_triple(nc, ps, ident, bhnl, e8, gxs4, base)
        for pi in range(BLK // 2):
            t0 = base + 2 * pi
            _superstep(nc, vs, wh, gxn4, tri, ha, hb, louts, t0, sbase + pi,
                       pre=lambda t=t0: _pre_triple(nc, ps, ident, bhnl, e8,
                                                    gxs4, t + 1))
            tri = _LAST_TRI[0]
            last = pi == BLK // 2 - 1
            _superstep(nc, vs, wh, gxn4, tri, hb, ha, louts, t0 + 1, None,
                       pre=(None if last else
                            lambda t=t0: _pre_triple(nc, ps, ident, bhnl, e8,
                                                     gxs4, t + 2)))
            if not last:
                tri = _LAST_TRI[0]


def build_nc(nblk1=16, nblk2=8):
    nc = bacc.Bacc("TRN2", target_bir_lowering=False, debug=False, num_devices=8)

    # ---------------- inputs ----------------
    xp_d = nc.dram_tensor("xp", [1031 * 130], BF, kind="ExternalInput")
    w1s_d = nc.dram_tensor("w1s", [128, 64], BF, kind="ExternalInput")
    w2s_d = nc.dram_tensor("w2s", [9, 128, 128], BF, kind="ExternalInput")
    w3s_d = nc.dram_tensor("w3s", [9, 128, 2, 128], BF, kind="ExternalInput")
    cb1_d = nc.dram_tensor("cb1", [64, 1], F32, kind="ExternalInput")
    sc1_d = nc.dram_tensor("sc1", [64, 1], F32, kind="ExternalInput")
    sh1_d = nc.dram_tensor("sh1", [64, 1], F32, kind="ExternalInput")
    cb2_d = nc.dram_tensor("cb2", [128, 1], F32, kind="ExternalInput")
    sc2_d = nc.dram_tensor("sc2", [128, 1], F32, kind="ExternalInput")
    sh2_d = nc.dram_tensor("sh2", [128, 1], F32, kind="ExternalInput")
    cb3_d = nc.dram_tensor("cb3", [128, 2], F32, kind="ExternalInput")
    sc3_d = nc.dram_tensor("sc3", [128, 2], F32, kind="ExternalInput")
    sh3_d = nc.dram_tensor("sh3", [128, 2], F32, kind="ExternalInput")
    wi1_d = nc.dram_tensor("wi1", [2, 12, 128, 32, 128], BF, kind="ExternalInput")
    gxb1_d = nc.dram_tensor("gxb1", [2, 128, 12], F32, kind="ExternalInput")
    wi2_d = nc.dram_tensor("wi2", [2, 128, 12 * 4 * 128], BF, kind="ExternalInput")
    gxb2_d = nc.dram_tensor("gxb2", [2, 128, 12], F32, kind="ExternalInput")
    wh1m_d = nc.dram_tensor("wh1m", [128, 4 * 12 * 128], BF, kind="ExternalInput")
    bhn1m_d = nc.dram_tensor("bhn1m", [128, 128], BF, kind="ExternalInput")
    wh2m_d = nc.dram_tensor("wh2m", [128, 4 * 12 * 128], BF, kind="ExternalInput")
    bhn2m_d = nc.dram_tensor("bhn2m", [128, 128], BF, kind="ExternalInput")
    ident_d = nc.dram_tensor("ident", [128, 128], BF, kind="ExternalInput")
    e8_d = nc.dram_tensor("e8", [128, 8], BF, kind="ExternalInput")
    dsel_d = nc.dram_tensor("dsel", [1, 1], U32, kind="ExternalInput")

    out_d = nc.dram_tensor("out", [128, 4, 256], F32, kind="ExternalOutput")
    yp_d = nc.dram_tensor("yp", [128, 32, 1024], BF, kind="Internal")

    with TileContext(nc) as tc:
      with tc.tile_pool(name="keep", bufs=1) as keep, \
           tc.tile_pool(name="arena1", bufs=1) as ar1, \
           tc.tile_pool(name="arena2", bufs=1) as ar2, \
           tc.tile_pool(name="scan_vs", bufs=6) as vs, \
           tc.tile_pool(name="dram", bufs=1, space="DRAM") as dram:
        # ---- collective bounce buffers
        ag1_in = dram.tile([2, 128, 12288], BF, name="ag1_in")
        ag1_out = dram.tile([4, 128, 12288], BF, name="ag1_out")
        ag2_in = dram.tile([2, 128, 2048], BF, name="ag2_in")
        ag2_out = dram.tile([4, 128, 2048], BF, name="ag2_out")
        ag3_in = dram.tile([2, 128, 6144], BF, name="ag3_in")
        ag3_out = dram.tile([4, 128, 6144], BF, name="ag3_out")
        ag4_in = dram.tile([2, 128, 1024], F32, name="ag4_in")
        ag4_out = dram.tile([4, 128, 1024], F32, name="ag4_out")

        sel_reg = nc.sync.alloc_register("sel_reg")
        nc.sync.reg_load(sel_reg, dsel_d[0:1, 0:1])
        sel = nc.sync.snap(sel_reg, min_val=0, max_val=1)

        # ---- long-lived small tiles
        gxb1_sb = []
        gxb2_sb = []
        for d in range(2):
            tg = keep.tile([128, 12], F32, tag=f"gxb1_{d}")
            nc.sync.dma_start(tg, gxb1_d[d])
            gxb1_sb.append(tg)
            tg = keep.tile([128, 12], F32, tag=f"gxb2_{d}")
            nc.sync.dma_start(tg, gxb2_d[d])
            gxb2_sb.append(tg)
        bhn1l = keep.tile([128, 128], BF, tag="bhn1l")
        nc.sync.dma_start(bhn1l, bhn1m_d[:])
        bhn2l = keep.tile([128, 128], BF, tag="bhn2l")
        nc.sync.dma_start(bhn2l, bhn2m_d[:])
        ident_sb = keep.tile([128, 128], BF, tag="ident")
        nc.sync.dma_start(ident_sb, ident_d[:])
        e8_sb = keep.tile([128, 8], BF, tag="e8")
        nc.sync.dma_start(e8_sb, e8_d[:])
        wh1 = keep.tile([128, 4 * 12 * 128], BF, tag="wh1")
        nc.sync.dma_start(wh1, wh1m_d[:])
        wh2 = keep.tile([128, 4 * 12 * 128], BF, tag="wh2")
        nc.sync.dma_start(wh2, wh2m_d[:])
        louts1 = keep.tile([128, 4, 2, 512], BF, tag="lo1", name="lo1")
        nc.vector.memset(louts1[:], 0.0)
        louts2 = keep.tile([128, 4, 2, 256], F32, tag="lo2", name="lo2")
        nc.vector.memset(louts2[:], 0.0)
        h1a = keep.tile([128, 8], BF, tag="h1a")
        h1b = keep.tile([128, 8], BF, tag="h1b")
        h2a = keep.tile([128, 8], BF, tag="h2a")
        h2b = keep.tile([128, 8], BF, tag="h2b")
        for h in (h1a, h1b, h2a, h2b):
            nc.vector.memset(h[:], 0.0)

        # ================== phase 1: convs ==================
        a1c = ar1.tile([128, X9_LEN + X3_LEN], BF, tag="ar1")
        x9 = a1c[:, 0:X9_LEN]
        x3 = a1c[:, X9_LEN:X9_LEN + X3_LEN]
        a2c = ar2.tile([128, X2_LEN + YPST_LEN], BF, tag="ar2")
        x2 = a2c[:, 0:X2_LEN]
        ypst = a2c[:, X2_LEN:X2_LEN + YPST_LEN]
        nc.vector.memset(a1c[:], 0.0)
        nc.vector.memset(a2c[:], 0.0)

        with tc.tile_pool(name="cw", bufs=1) as cw, \
             tc.tile_pool(name="p1psum", bufs=4, space="PSUM") as pp1, \
             tc.tile_pool(name="p1tmp", bufs=3) as tp1:
            w1s = cw.tile([128, 64], BF)
            nc.sync.dma_start(w1s, w1s_d[:])
            w2s = cw.tile([128, 9 * 128], BF)
            nc.sync.dma_start(w2s[:].rearrange("p (s j) -> p s j", s=9),
                              w2s_d[:].rearrange("s p j -> p s j"))
            w3s = cw.tile([128, 9 * 2 * 128], BF)
            nc.sync.dma_start(
                w3s[:].rearrange("p (s c j) -> p s c j", s=9, c=2),
                w3s_d[:].rearrange("s p c j -> p s c j"))
            cb1 = cw.tile([64, 1], F32)
            nc.sync.dma_start(cb1, cb1_d[:])
            sc1 = cw.tile([64, 1], F32)
            nc.sync.dma_start(sc1, sc1_d[:])
            sh1 = cw.tile([64, 1], F32)
            nc.sync.dma_start(sh1, sh1_d[:])
            cb2 = cw.tile([128, 1], F32)
            nc.sync.dma_start(cb2, cb2_d[:])
            sc2 = cw.tile([128, 1], F32)
            nc.sync.dma_start(sc2, sc2_d[:])
            sh2 = cw.tile([128, 1], F32)
            nc.sync.dma_start(sh2, sh2_d[:])
            cb3 = cw.tile([128, 2], F32)
            nc.sync.dma_start(cb3, cb3_d[:])
            sc3 = cw.tile([128, 2], F32)
            nc.sync.dma_start(sc3, sc3_d[:])
            sh3 = cw.tile([128, 2], F32)
            nc.sync.dma_start(sh3, sh3_d[:])

            for i in range(8):
                t0 = i * 128
                for dh in range(3):
                    for dw in range(3):
                        s = dh * 3 + dw
                        start = (t0 + dh) * 130 + dw
                        nc.sync.dma_start(x9[s:s + 1, 0:132 * 130],
                                          xp_d[ds(start, 132 * 130)][None, :])
                # ---- conv1: 33 chunks of (4 rows x 128 f)
                for c in range(33):
                    psum = pp1.tile([128, 512], F32, tag="cpsum")
                    rhs = x9[:, c * 520:c * 520 + 520].rearrange(
                        "p (r w) -> p r w", w=130)[:, :, 0:128]
                    nc.tensor.matmul(psum[0:64], w1s, rhs, start=True, stop=True)
                    tmp = tp1.tile([64, 512], BF, tag="c1tmp")
                    nc.scalar.activation(tmp, psum[0:64], AF.Relu, bias=cb1)
                    tr = tmp[:].rearrange("q (r f e) -> q r f e", f=64, e=2)
                    pm = tp1.tile([64, 256], BF, tag="c1pm")
                    pmr = pm[:].rearrange("q (r f) -> q r f", f=64)
                    nc.vector.tensor_tensor(pmr, tr[:, :, :, 0], tr[:, :, :, 1],
                                            OP.max)
                    xv = x2[0:64, c * 264:c * 264 + 264].rearrange(
                        "q (r w) -> q r w", w=66)[:, :, 1:65]
                    nc.vector.scalar_tensor_tensor(
                        xv, pmr, sc1, sh1[:, 0:1, None].to_broadcast(pmr.shape),
                        OP.mult, OP.add)
                if i == 0:
                    nc.vector.memset(x2[0:64, 0:132], 0.0)
                if i == 7:
                    nc.vector.memset(x2[0:64, 130 * 66:132 * 66], 0.0)
                # ---- conv2: 17 chunks of (<=8 rows x 64 f)
                for c in range(17):
                    r0 = c * 8
                    rows = min(8, 130 - r0)
                    nfree = rows * 64
                    psum = pp1.tile([128, 512], F32, tag="cpsum")
                    for si in range(9):
                        dh, dw = si // 3, si % 3
                        off = (r0 + dh) * 66 + dw
                        rhs = x2[:, off:off + rows * 66].rearrange(
                            "p (r w) -> p r w", w=66)[:, :, 0:64]
                        nc.tensor.matmul(psum[:, 0:nfree],
                                         w2s[:, si * 128:(si + 1) * 128],
                                         rhs, start=(si == 0), stop=(si == 8))
                    tmp = tp1.tile([128, 512], BF, tag="c2tmp")
                    nc.scalar.activation(tmp[:, 0:nfree], psum[:, 0:nfree],
                                         AF.Relu, bias=cb2)
                    tr = tmp[:, 0:nfree].rearrange("p (r f e) -> p r f e",
                                                   f=32, e=2)
                    pm = tp1.tile([128, 256], BF, tag="c2pm")
                    pmr = pm[:, 0:rows * 32].rearrange("p (r f) -> p r f", f=32)
                    nc.vector.tensor_tensor(pmr, tr[:, :, :, 0], tr[:, :, :, 1],
                                            OP.max)
                    xv = x3[:, r0 * 34:r0 * 34 + rows * 34].rearrange(
                        "p (r w) -> p r w", w=34)[:, :, 1:33]
                    nc.vector.scalar_tensor_tensor(
                        xv, pmr, sc2, sh2[:, 0:1, None].to_broadcast(pmr.shape),
                        OP.mult, OP.add)
                if i == 0:
                    nc.vector.memset(x3[:, 0:34], 0.0)
                if i == 7:
                    nc.vector.memset(x3[:, 129 * 34:130 * 34], 0.0)
                # ---- conv3: 2 co-chunks x 8 chunks of (16 rows x 32 f)
                for ch in range(2):
                    for c in range(8):
                        r0 = c * 16
                        psum = pp1.tile([128, 512], F32, tag="cpsum")
                        for si in range(9):
                            dh, dw = si // 3, si % 3
                            off = (r0 + dh) * 34 + dw
                            rhs = x3[:, off:off + 16 * 34].rearrange(
                                "p (r w) -> p r w", w=34)[:, :, 0:32]
                            nc.tensor.matmul(
                                psum,
                                w3s[:, (si * 2 + ch) * 128:(si * 2 + ch + 1) * 128],
                                rhs, start=(si == 0), stop=(si == 8))
                        tmp = tp1.tile([128, 512], BF, tag="c3tmp")
                        nc.scalar.activation(tmp, psum, AF.Relu,
                                             bias=cb3[:, ch:ch + 1])
                        tr = tmp[:].rearrange("p (r f e) -> p f r e", f=16, e=2)
                        pm = tp1.tile([128, 256], BF, tag="c3pm")
                        pmr = pm[:].rearrange("p (f r) -> p f r", r=16)
                        nc.vector.tensor_tensor(pmr, tr[:, :, :, 0],
                                                tr[:, :, :, 1], OP.max)
                        yv = ypst[:].rearrange("p (f c t) -> p f c t",
                                               f=16, c=2)[:, :, ch, r0:r0 + 16]
                        nc.vector.scalar_tensor_tensor(
                            yv, pmr, sc3[:, ch:ch + 1],
                            sh3[:, ch:ch + 1, None].to_broadcast(pmr.shape),
                            OP.mult, OP.add)
                nc.sync.dma_start(yp_d[:, :, ds(t0, 128)],
                                  ypst[:].rearrange("p (k t) -> p k t", k=32))

        # ================== phase 2: gx1 ==================
        # full yp resident -> one 32-deep psum accumulation per (d, mc, tch),
        # bias applied on the psum->bf16 store; per-dir pair-AG fired as soon
        # as that direction is staged (fwd AG hides under bwd compute)
        a2t = ar2.tile([128, 2 * 12288], BF, tag="ar2")
        gxst = [a2t[:, 0:12288].rearrange("p (m t) -> p m t", t=1024),
                a2t[:, 12288:].rearrange("p (m t) -> p m t", t=1024)]
        ypsb_t = ar1.tile([128, 32 * 1024], BF, tag="ar1")
        ypsb = ypsb_t[:].rearrange("p (k t) -> p k t", t=1024)
        nc.sync.dma_start(ypsb, yp_d[:])
        with tc.tile_pool(name="wi1sb", bufs=2) as wip, \
             tc.tile_pool(name="p2psum", bufs=4, space="PSUM") as pp2:
            for d in range(2):
                for mc in range(12):
                    wisb = wip.tile([128, 32 * 128], BF, tag="wi1t")
                    nc.sync.dma_start(
                        wisb[:].rearrange("p (k j) -> p k j", k=32),
                        wi1_d[d, mc])
                    for tch in range(2):
                        psum = pp2.tile([128, 512], F32, tag="gxpsum")
                        for kc in range(32):
                            nc.tensor.matmul(
                                psum, wisb[:, kc * 128:(kc + 1) * 128],
                                ypsb[:, kc, ds(tch * 512, 512)],
                                start=(kc == 0), stop=(kc == 31))
                        if d == 1 and NEG_STRIDE_GX:
                            gview = gxst[d][:, mc, 1023 - tch * 512::-1][:, 0:512]
                        else:
                            gview = gxst[d][:, mc, tch * 512:(tch + 1) * 512]
                        nc.vector.tensor_scalar_add(
                            gview, psum, gxb1_sb[d][:, mc:mc + 1])
                nc.sync.dma_start(
                    ag1_in[d],
                    gxst[d].rearrange("p m t -> p (m t)"))
                nc.gpsimd.collective_compute(
                    "AllGather", OP.bypass, replica_groups=PAIR_GROUPS,
                    ins=[ag1_in[d][None, :].opt()],
                    outs=[ag1_out[:].rearrange(
                        "(d s) p f -> d s p f", d=2)[d].opt()])

        # my two chains: dir-major gathered slots {2*sel, 2*sel+1}
        gxs1_t = ar1.tile([128, 12 * 2 * 1024], BF, tag="ar1")
        gxs1 = gxs1_t[:].rearrange("p (m c t) -> p m c t", c=2, t=1024)
        ag1v = ag1_out[:].rearrange("s p f -> p s f")
        for ch in range(2):
            nc.sync.dma_start(gxs1[:, :, ch:ch + 1, :],
                              ag1v[:, ds(2 * sel + ch, 1), :].rearrange(
                                  "p s (m t) -> p m s t", t=1024))

        # ================== phase 3: L1 scans ==================
        ps_ctx = tc.tile_pool(name="scan_ps", bufs=2, space="PSUM")
        ps = ps_ctx.__enter__()
        _scan_loop(nc, tc, vs, ps, nblk1, wh1, gxs1, gxs1[:, 8:12],
                   ident_sb, bhn1l, e8_sb, h1a, h1b, louts1)

        # louts exchange: pairwise AG by chain -> my sample's fwd+bwd
        for ch in range(2):
            nc.sync.dma_start(ag2_in[ch], louts1[:, :, ch, :])
        nc.gpsimd.collective_compute(
            "AllGather", OP.bypass, replica_groups=PAIR_GROUPS,
            ins=[ag2_in[:].opt()], outs=[ag2_out[:].opt()])
        l2in = keep.tile([128, 4 * 512], BF, tag="l2in")
        with tc.tile_pool(name="mid1", bufs=1) as mid1:
            l2pair = mid1.tile([128, 2, 2048], BF, tag="l2pair")
            ag2v = ag2_out[:].rearrange("s p f -> p s f")
            for ch in range(2):
                nc.sync.dma_start(l2pair[:, ch:ch + 1, :],
                                  ag2v[:, ds(sel + 2 * ch, 1), :])
            nc.vector.tensor_tensor(l2in, l2pair[:, 0, :], l2pair[:, 1, :],
                                    OP.add)

        # ================== phase 4: gx2 (own sample, both dirs) ==========
        gx2f_t = ar1.tile([128, 12288], F32, tag="ar1")
        gx2f = gx2f_t[:, 0:6144].rearrange("p (m t) -> p m t", t=512)
        gx2b = gx2f_t[:, 6144:12288].rearrange("p (m t) -> p m t", t=512)
        gx2t = (gx2f, gx2b)
        wi2sb = []
        w4 = ar2.tile([128, 12288], BF, tag="ar2")
        wi2sb = [w4[:, 0:6144], w4[:, 6144:12288]]
        for d in range(2):
            nc.sync.dma_start(wi2sb[d], wi2_d[d])
        with tc.tile_pool(name="p4psum", bufs=2, space="PSUM") as pp4:
            for d in range(2):
                for mc in range(12):
                    psum = pp4.tile([128, 512], F32, tag="gx2psum")
                    for kc in range(4):
                        nc.tensor.matmul(
                            psum,
                            wi2sb[d][:, (mc * 4 + kc) * 128:(mc * 4 + kc + 1) * 128],
                            l2in[:, kc * 512:(kc + 1) * 512],
                            start=(kc == 0), stop=(kc == 3))
                    if d == 1 and NEG_STRIDE_GX:
                        gview = gx2t[d][:, mc, 511::-1][:, 0:512]
                    else:
                        gview = gx2t[d][:, mc, :]
                    nc.vector.tensor_scalar_add(gview, psum,
                                                gxb2_sb[d][:, mc:mc + 1])
        with tc.tile_pool(name="gx2cvt", bufs=2) as gcp2:
            for d in range(2):
                st = gcp2.tile([128, 6144], BF, tag="gx2cvt")
                nc.vector.tensor_copy(st, gx2f_t[:, d * 6144:(d + 1) * 6144])
                nc.sync.dma_start(ag3_in[d], st[:])
        nc.gpsimd.collective_compute(
            "AllGather", OP.bypass, replica_groups=PAIR_GROUPS,
            ins=[ag3_in[:].opt()], outs=[ag3_out[:].opt()])
        gxs2_t = ar2.tile([128, 12 * 2 * 512], BF, tag="ar2")
        gxs2 = gxs2_t[:].rearrange("p (m c t) -> p m c t", c=2, t=512)
        ag3v = ag3_out[:].rearrange("s p f -> p s f")
        for ch in range(2):
            nc.sync.dma_start(gxs2[:, :, ch:ch + 1, :],
                              ag3v[:, ds(sel + 2 * ch, 1), :].rearrange(
                                  "p s (m t) -> p m s t", t=512))

        # ================== phase 5: L2 scans ==================
        _scan_loop(nc, tc, vs, ps, nblk2, wh2, gxs2, gxs2[:, 8:12],
                   ident_sb, bhn2l, e8_sb, h2a, h2b, louts2)

        ps_ctx.__exit__(None, None, None)

        # final: pairwise AG of partial outputs by chain, sum own sample
        for ch in range(2):
            nc.sync.dma_start(ag4_in[ch], louts2[:, :, ch, :])
        nc.gpsimd.collective_compute(
            "AllGather", OP.bypass, replica_groups=PAIR_GROUPS,
            ins=[ag4_in[:].opt()], outs=[ag4_out[:].opt()])
        with tc.tile_pool(name="fin", bufs=1) as fin:
            opair = fin.tile([128, 2, 1024], F32, tag="opair")
            ag4v = ag4_out[:].rearrange("s p f -> p s f")
            for ch in range(2):
                nc.sync.dma_start(opair[:, ch:ch + 1, :],
                                  ag4v[:, ds(sel + 2 * ch, 1), :])
            osb = fin.tile([128, 4 * 256], F32, tag="osb")
            nc.vector.tensor_tensor(osb, opair[:, 0, :], opair[:, 1, :],
                                    OP.add)
            nc.sync.dma_start(out_d[:],
                              osb[:].rearrange("p (k s) -> p k s", k=4))

    nc.compile()
    return nc


# --------------------------------------------------------------------------
# host-side preprocessing
# --------------------------------------------------------------------------

def _bn(g, be, rm, rv):
    s = np.asarray(g) / np.sqrt(np.asarray(rv) + BN_EPS)
    return (s.astype(np.float32),
            (np.asarray(be) - np.asarray(rm) * s).astype(np.float32))


def _prep_common(inputs):
    d = {}
    cw1 = np.asarray(inputs['cw1'])
    w1s = np.zeros((128, 64), np.float32)
    for dh in range(3):
        for dw in range(3):
            w1s[dh * 3 + dw] = cw1[:, 0, dh, dw]
    d['w1s'] = w1s.astype(BF16)
    w2 = np.asarray(inputs['cw2'])
    w2s = np.zeros((9, 128, 128), np.float32)
    w2s[:, 0:64, :] = w2.transpose(2, 3, 1, 0).reshape(9, 64, 128)
    d['w2s'] = w2s.astype(BF16)
    w3 = np.asarray(inputs['cw3'])
    d['w3s'] = np.ascontiguousarray(
        w3.transpose(2, 3, 1, 0).reshape(9, 128, 2, 128)).astype(BF16)
    sc1, sh1 = _bn(inputs['g1'], inputs['be1'], inputs['rm1'], inputs['rv1'])
    sc2, sh2 = _bn(inputs['g2'], inputs['be2'], inputs['rm2'], inputs['rv2'])
    sc3, sh3 = _bn(inputs['g3'], inputs['be3'], inputs['rm3'], inputs['rv3'])
    d['cb1'] = np.asarray(inputs['cb1'], np.float32).reshape(64, 1)
    d['sc1'] = sc1.reshape(64, 1)
    d['sh1'] = sh1.reshape(64, 1)
    d['cb2'] = np.asarray(inputs['cb2'], np.float32).reshape(128, 1)
    d['sc2'] = sc2.reshape(128, 1)
    d['sh2'] = sh2.reshape(128, 1)
    d['cb3'] = np.ascontiguousarray(
        np.asarray(inputs['cb3'], np.float32).reshape(2, 128).T)
    d['sc3'] = np.ascontiguousarray(sc3.reshape(2, 128).T)
    d['sh3'] = np.ascontiguousarray(sh3.reshape(2, 128).T)

    dprime = np.arange(4096)
    perm = (dprime % 256) * 16 + dprime // 256

    wi1 = np.zeros((2, 12, 128, 32, 128), np.float32)
    gxb1 = np.zeros((2, 128, 12), np.float32)
    wh1 = np.zeros((2, 128, 4 * 12 * 128), np.float32)
    bhn1 = np.zeros((2, 128, 4), np.float32)
    wi2 = np.zeros((2, 128, 12 * 4 * 128), np.float32)
    gxb2 = np.zeros((2, 128, 12), np.float32)
    wh2 = np.zeros((2, 128, 4 * 12 * 128), np.float32)
    bhn2 = np.zeros((2, 128, 4), np.float32)
    for di, tag in enumerate('fb'):
        wi = np.asarray(inputs[f'wi{tag}1'])[:, perm]
        wi1[di] = wi.reshape(12, 128, 32, 128).transpose(0, 3, 2, 1)
        bias = np.asarray(inputs[f'bi{tag}1']).copy()
        bh = np.asarray(inputs[f'bh{tag}1'])
        bias[:1024] += bh[:1024]
        gxb1[di] = bias.reshape(12, 128).T
        wh1[di] = np.asarray(inputs[f'wh{tag}1']).reshape(
            12, 128, 4, 128).transpose(3, 2, 0, 1).reshape(128, -1)
        bhn1[di] = bh[1024:].reshape(4, 128).T
        wi2v = np.asarray(inputs[f'wi{tag}2'])
        wi2[di] = wi2v.reshape(12, 128, 4, 128).transpose(
            3, 0, 2, 1).reshape(128, -1)
        bias2 = np.asarray(inputs[f'bi{tag}2']).copy()
        bh2 = np.asarray(inputs[f'bh{tag}2'])
        bias2[:1024] += bh2[:1024]
        gxb2[di] = bias2.reshape(12, 128).T
        wh2[di] = np.asarray(inputs[f'wh{tag}2']).reshape(
            12, 128, 4, 128).transpose(3, 2, 0, 1).reshape(128, -1)
        bhn2[di] = bh2[1024:].reshape(4, 128).T
    d['wi1'] = wi1.astype(BF16)
    d['gxb1'] = gxb1
    d['wi2'] = wi2.astype(BF16)
    d['gxb2'] = gxb2
    d['_wh1'] = wh1.astype(BF16)
    d['_bhn1'] = bhn1
    d['_wh2'] = wh2.astype(BF16)
    d['_bhn2'] = bhn2
    return d


def _bhn_lhst(bhn):
    # lhsT[c, p] = bhn8[p, c] for c < 8 (out[p, c] = lhsT[c, p] under rhs=e8)
    bhn8 = np.repeat(bhn, 2, axis=1)          # [128, 8] (j c)
    lhst = np.zeros((128, 128), np.float32)
    lhst[0:8, :] = bhn8.T
    return lhst.astype(BF16)


def _prep_sample(x_c):
    xp = np.zeros((1031, 130), np.float32)
    xp[3:1027, 1:129] = x_c
    return {'xp': xp.astype(BF16).reshape(-1)}


def get_nc(nblk1=16, nblk2=8):
    key = (nblk1, nblk2)
    if key not in _CACHED_NC:
        _CACHED_NC[key] = build_nc(nblk1, nblk2)
    return _CACHED_NC[key]


def run(inputs, nblk1=16, nblk2=8, **kwargs):
    nc = get_nc(nblk1, nblk2)
    common = _prep_common(inputs)
    private = {k: common.pop(k) for k in list(common) if k.startswith('_')}
    x = np.asarray(inputs['x'])
    in_maps = []
    for c in range(8):
        m = dict(common)
        m.update(_prep_sample(x[c, 0]))
        par = c % 2
        m['wh1m'] = private['_wh1'][par]
        m['bhn1m'] = _bhn_lhst(private['_bhn1'][par])
        m['wh2m'] = private['_wh2'][par]
        m['bhn2m'] = _bhn_lhst(private['_bhn2'][par])
        m['ident'] = np.eye(128, dtype=BF16)
        m['e8'] = np.eye(128, dtype=BF16)[:, :8].copy()
        m['dsel'] = np.array([[par]], dtype=np.uint32)
        in_maps.append(m)
    return run_bass_kernel_spmd(nc, in_maps, core_ids=list(range(8)), **kwargs)


def kernel(**inputs) -> np.ndarray:
    res = run(inputs)
    outs = []
    for c in range(8):
        o = np.asarray(res.results[c]['out'])  # [128, 4, 256]
        outs.append(np.ascontiguousarray(
            o.transpose(2, 1, 0).reshape(256, 512)))
    return np.stack(outs).astype(np.float32)


# revision 18
# speedup vs baseline: 1.1821x; 1.1821x over previous
"""Trainium2 Bass kernel for nn_CategoryBranch2 (3 conv blocks + 2 BiGRU layers).

Distribution:
  phase 1/2 (convs + gx1) data-parallel: one sample per core (8 cores).
  GRU scans regrouped so each core scans TWO SAME-DIRECTION chains (the two
  samples of its core-pair): even cores scan forward chains, odd cores scan
  backward chains.  The two chains share the recurrent weight matrix, so each
  scan step is 48 matmuls of [128cont x 128out x 2cols] instead of 96 1-col
  matmuls -- half the PE instruction stream, the real bottleneck.

  Regrouping uses pairwise AllGather collectives (groups [0,1],[2,3],...) on
  internal DRAM bounce buffers; each core picks its two chains out of the
  4 gathered slots with one dynamic-offset DMA (slot index is a per-core
  uint32 input 'dsel' = core%2; chains sit at slots {dsel, dsel+2}).

  Pipeline: convs -> gx1 -> AG1(gx1 by dir) -> L1 scans -> AG2(louts by
  sample) -> l2in -> gx2 -> AG3(gx2 by dir) -> L2 scans -> AG4(partials by
  sample) -> sum -> out.  All collectives are pair-local.

Device plan per core (phases 1-2 identical to the data-parallel baseline):
  phase1  conv blocks, T-tiled (8 tiles of 128 steps, halo recompute).
  phase2  gx1[dir] = wi1' @ y' + (bi + bh_rz), fp32 SBUF, bwd stored
          time-reversed; converted to bf16 for the collective.

Self-contained: hardcodes all shapes; host does only numpy weight re-layouts.
"""

import numpy as np
import ml_dtypes

import concourse.bacc as bacc
import concourse.bass as bass
import concourse.mybir as mybir
from concourse.tile import TileContext
from concourse.bass import ds
from concourse.bass_utils import run_bass_kernel_spmd

BF16 = ml_dtypes.bfloat16
F32 = mybir.dt.float32
BF = mybir.dt.bfloat16
U32 = mybir.dt.uint32
AF = mybir.ActivationFunctionType
OP = mybir.AluOpType
PE = mybir.EngineType.PE

BN_EPS = 1e-5
NEG_STRIDE_GX = True   # write bwd gx time-reversed (negative-stride DVE store)

X9_LEN = 134 * 130          # 17420
X2_LEN = 132 * 66 + 66      # 8778
X3_LEN = 130 * 34 + 34      # 4454
YPST_LEN = 32 * 128         # 4096

PAIR_GROUPS = [[0, 1], [2, 3], [4, 5], [6, 7]]

_CACHED_NC = {}


def _pre_triple(nc, ps, ident, bhnl, e8, gxs4, t):
    """Allocate the psum triple for step t and preload it via PE matmuls:
    psr/psz <- gx_rz(t) (identity-stationary copy), psn <- bhn (one-hot rhs).
    Emitted in the PREVIOUS superstep's matmul stream, so the DVE queue stays
    clean for the q->npre->b->f critical chain."""
    psr = ps.tile([128, 8], F32, tag="psr", name="psr")
    psz = ps.tile([128, 8], F32, tag="psz", name="psz")
    psn = ps.tile([128, 8], F32, tag="psn", name="psn")
    nc.tensor.matmul(psn, bhnl, e8, start=True, stop=False,
                     skip_group_check=True)
    nc.tensor.matmul(psr, ident, gxs4[:, 0:4, :, ds(t, 1)],
                     start=True, stop=False, skip_group_check=True)
    nc.tensor.matmul(psz, ident, gxs4[:, 4:8, :, ds(t, 1)],
                     start=True, stop=False, skip_group_check=True)
    return psr, psz, psn


_LAST_TRI = [None]


def _superstep(nc, vs, wh, gxn4, tri, h_old, h_new, louts, t, s_out, pre=None):
    """One time step for TWO same-direction chains (matmul columns).

    Per-gate psum tiles (r/z/n, [128, 8], col = j*2 + ch), preloaded by
    _pre_triple.  Matmul order n -> r -> z so psn's PE->DVE semaphore and the
    r-sigmoid both resolve as early as possible.  All tail ops flat [128, 8].
    h_* [128, 8] bf16: col = kc*2 + ch.
    """
    psr, psz, psn = tri
    for g, pst in ((2, psn), (0, psr), (1, psz)):
        for j in range(4):
            mc = g * 4 + j
            for kc in range(4):
                nc.tensor.matmul(
                    pst[:, j * 2:j * 2 + 2],
                    wh[:, (kc * 12 + mc) * 128:(kc * 12 + mc + 1) * 128],
                    h_old[:, kc * 2:kc * 2 + 2],
                    start=False, stop=(kc == 3), skip_group_check=True)
    if pre is not None:
        _LAST_TRI[0] = pre()
    # tail: sig_r -> q -> npre -> tanh_n -> b -> f -> tanh_h is the serial
    # chain; the z path (sig_z, zc, a) runs beside it on ACT/Pool.
    r = vs.tile([128, 8], F32, tag="r")
    nc.scalar.activation(r, psr, AF.Sigmoid)
    z = vs.tile([128, 8], F32, tag="z")
    nc.scalar.activation(z, psz, AF.Sigmoid)
    q = vs.tile([128, 8], F32, tag="q")
    nc.vector.tensor_tensor(q, r, psn, OP.mult)
    npre = vs.tile([128, 8], F32, tag="npre")
    nc.vector.tensor_tensor(npre[:].rearrange("p (j c) -> p j c", c=2)[:, :, :, None],
                            q[:].rearrange("p (j c) -> p j c", c=2)[:, :, :, None],
                            gxn4[:, :, :, ds(t, 1)], OP.add)
    zc = vs.tile([128, 8], F32, tag="zc")
    nc.gpsimd.tensor_scalar(zc, z, -1.0, 1.0, OP.mult, OP.add)
    a = vs.tile([128, 8], F32, tag="a")
    nc.gpsimd.tensor_mul(a, z, h_old[:])
    n = vs.tile([128, 8], BF, tag="n")
    nc.scalar.activation(n, npre, AF.Tanh)
    b = vs.tile([128, 8], F32, tag="b")
    nc.vector.tensor_tensor(b, zc, n, OP.mult)
    f = vs.tile([128, 8], F32, tag="f")
    nc.vector.tensor_tensor(f, a, b, OP.add)
    nc.scalar.activation(h_new[:], f, AF.Tanh)
    if s_out is not None:
        nc.gpsimd.tensor_copy(
            louts[:, :, :, ds(s_out, 1)],
            h_new[:].rearrange("p (k c) -> p k c", c=2)[:, :, :, None])


def _scan_loop(nc, tc, vs, ps, nblk, wh, gxs4, gxn4, ident, bhnl, e8,
               ha, hb, louts, BLK=64):
    with tc.For_i(0, nblk, 1, hint_engines=(PE,)) as blk:
        base = blk * BLK
        sbase = blk * (BLK // 2)
        tri = _pre_triple(nc, ps, ident, bhnl, e8, gxs4, base)
        for pi in range(BLK // 2):
            t0 = base + 2 * pi
            _superstep(nc, vs, wh, gxn4, tri, ha, hb, louts, t0, sbase + pi,
                       pre=lambda t=t0: _pre_triple(nc, ps, ident, bhnl, e8,
                                                    gxs4, t + 1))
            tri = _LAST_TRI[0]
            last = pi == BLK // 2 - 1
            _superstep(nc, vs, wh, gxn4, tri, hb, ha, louts, t0 + 1, None,
                       pre=(None if last else
                            lambda t=t0: _pre_triple(nc, ps, ident, bhnl, e8,
                                                     gxs4, t + 2)))
            if not last:
                tri = _LAST_TRI[0]


def build_nc(nblk1=16, nblk2=8):
    nc = bacc.Bacc("TRN2", target_bir_lowering=False, debug=False, num_devices=8)

    # ---------------- inputs ----------------
    xp_d = nc.dram_tensor("xp", [1031 * 130], BF, kind="ExternalInput")
    w1s_d = nc.dram_tensor("w1s", [128, 64], BF, kind="ExternalInput")
    w2s_d = nc.dram_tensor("w2s", [9, 128, 128], BF, kind="ExternalInput")
    w3s_d = nc.dram_tensor("w3s", [9, 128, 2, 128], BF, kind="ExternalInput")
    cb1_d = nc.dram_tensor("cb1", [64, 1], F32, kind="ExternalInput")
    sc1_d = nc.dram_tensor("sc1", [64, 1], F32, kind="ExternalInput")
    sh1_d = nc.dram_tensor("sh1", [64, 1], F32, kind="ExternalInput")
    cb2_d = nc.dram_tensor("cb2", [128, 1], F32, kind="ExternalInput")
    sc2_d = nc.dram_tensor("sc2", [128, 1], F32, kind="ExternalInput")
    sh2_d = nc.dram_tensor("sh2", [128, 1], F32, kind="ExternalInput")
    cb3_d = nc.dram_tensor("cb3", [128, 2], F32, kind="ExternalInput")
    sc3_d = nc.dram_tensor("sc3", [128, 2], F32, kind="ExternalInput")
    sh3_d = nc.dram_tensor("sh3", [128, 2], F32, kind="ExternalInput")
    wi1_d = nc.dram_tensor("wi1", [2, 12, 128, 32, 128], BF, kind="ExternalInput")
    gxb1_d = nc.dram_tensor("gxb1", [2, 128, 12], F32, kind="ExternalInput")
    wi2_d = nc.dram_tensor("wi2", [2, 128, 12 * 4 * 128], BF, kind="ExternalInput")
    gxb2_d = nc.dram_tensor("gxb2", [2, 128, 12], F32, kind="ExternalInput")
    wh1m_d = nc.dram_tensor("wh1m", [128, 4 * 12 * 128], BF, kind="ExternalInput")
    bhn1m_d = nc.dram_tensor("bhn1m", [128, 128], BF, kind="ExternalInput")
    wh2m_d = nc.dram_tensor("wh2m", [128, 4 * 12 * 128], BF, kind="ExternalInput")
    bhn2m_d = nc.dram_tensor("bhn2m", [128, 128], BF, kind="ExternalInput")
    ident_d = nc.dram_tensor("ident", [128, 128], BF, kind="ExternalInput")
    e8_d = nc.dram_tensor("e8", [128, 8], BF, kind="ExternalInput")
    dsel_d = nc.dram_tensor("dsel", [1, 1], U32, kind="ExternalInput")

    out_d = nc.dram_tensor("out", [128, 4, 256], F32, kind="ExternalOutput")
    yp_d = nc.dram_tensor("yp", [128, 32, 1024], BF, kind="Internal")

    with TileContext(nc) as tc:
      with tc.tile_pool(name="keep", bufs=1) as keep, \
           tc.tile_pool(name="arena1", bufs=1) as ar1, \
           tc.tile_pool(name="arena2", bufs=1) as ar2, \
           tc.tile_pool(name="scan_vs", bufs=6) as vs, \
           tc.tile_pool(name="dram", bufs=1, space="DRAM") as dram:
        # ---- collective bounce buffers
        ag1_in = dram.tile([2, 128, 12288], BF, name="ag1_in")
        ag1_out = dram.tile([4, 128, 12288], BF, name="ag1_out")
        ag2_in = dram.tile([2, 128, 2048], BF, name="ag2_in")
        ag2_out = dram.tile([4, 128, 2048], BF, name="ag2_out")
        ag3_in = dram.tile([2, 128, 6144], BF, name="ag3_in")
        ag3_out = dram.tile([4, 128, 6144], BF, name="ag3_out")
        ag4_in = dram.tile([2, 128, 1024], F32, name="ag4_in")
        ag4_out = dram.tile([4, 128, 1024], F32, name="ag4_out")

        sel_reg = nc.sync.alloc_register("sel_reg")
        nc.sync.reg_load(sel_reg, dsel_d[0:1, 0:1])
        sel = nc.sync.snap(sel_reg, min_val=0, max_val=1)

        # ---- long-lived small tiles
        gxb1_sb = []
        gxb2_sb = []
        for d in range(2):
            tg = keep.tile([128, 12], F32, tag=f"gxb1_{d}")
            nc.sync.dma_start(tg, gxb1_d[d])
            gxb1_sb.append(tg)
            tg = keep.tile([128, 12], F32, tag=f"gxb2_{d}")
            nc.sync.dma_start(tg, gxb2_d[d])
            gxb2_sb.append(tg)
        bhn1l = keep.tile([128, 128], BF, tag="bhn1l")
        nc.sync.dma_start(bhn1l, bhn1m_d[:])
        bhn2l = keep.tile([128, 128], BF, tag="bhn2l")
        nc.sync.dma_start(bhn2l, bhn2m_d[:])
        ident_sb = keep.tile([128, 128], BF, tag="ident")
        nc.sync.dma_start(ident_sb, ident_d[:])
        e8_sb = keep.tile([128, 8], BF, tag="e8")
        nc.sync.dma_start(e8_sb, e8_d[:])
        wh1 = keep.tile([128, 4 * 12 * 128], BF, tag="wh1")
        nc.sync.dma_start(wh1, wh1m_d[:])
        wh2 = keep.tile([128, 4 * 12 * 128], BF, tag="wh2")
        nc.sync.dma_start(wh2, wh2m_d[:])
        louts1 = keep.tile([128, 4, 2, 512], BF, tag="lo1", name="lo1")
        nc.vector.memset(louts1[:], 0.0)
        louts2 = keep.tile([128, 4, 2, 256], F32, tag="lo2", name="lo2")
        nc.vector.memset(louts2[:], 0.0)
        h1a = keep.tile([128, 8], BF, tag="h1a")
        h1b = keep.tile([128, 8], BF, tag="h1b")
        h2a = keep.tile([128, 8], BF, tag="h2a")
        h2b = keep.tile([128, 8], BF, tag="h2b")
        for h in (h1a, h1b, h2a, h2b):
            nc.vector.memset(h[:], 0.0)

        # ================== phase 1: convs ==================
        a1c = ar1.tile([128, X9_LEN + X3_LEN], BF, tag="ar1")
        x9 = a1c[:, 0:X9_LEN]
        x3 = a1c[:, X9_LEN:X9_LEN + X3_LEN]
        a2c = ar2.tile([128, X2_LEN + YPST_LEN], BF, tag="ar2")
        x2 = a2c[:, 0:X2_LEN]
        ypst = a2c[:, X2_LEN:X2_LEN + YPST_LEN]
        nc.vector.memset(a1c[:], 0.0)
        nc.vector.memset(a2c[:], 0.0)

        with tc.tile_pool(name="cw", bufs=1) as cw, \
             tc.tile_pool(name="p1psum", bufs=4, space="PSUM") as pp1, \
             tc.tile_pool(name="p1tmp", bufs=3) as tp1:
            w1s = cw.tile([128, 64], BF)
            nc.sync.dma_start(w1s, w1s_d[:])
            w2s = cw.tile([128, 9 * 128], BF)
            nc.sync.dma_start(w2s[:].rearrange("p (s j) -> p s j", s=9),
                              w2s_d[:].rearrange("s p j -> p s j"))
            w3s = cw.tile([128, 9 * 2 * 128], BF)
            nc.sync.dma_start(
                w3s[:].rearrange("p (s c j) -> p s c j", s=9, c=2),
                w3s_d[:].rearrange("s p c j -> p s c j"))
            cb1 = cw.tile([64, 1], F32)
            nc.sync.dma_start(cb1, cb1_d[:])
            sc1 = cw.tile([64, 1], F32)
            nc.sync.dma_start(sc1, sc1_d[:])
            sh1 = cw.tile([64, 1], F32)
            nc.sync.dma_start(sh1, sh1_d[:])
            cb2 = cw.tile([128, 1], F32)
            nc.sync.dma_start(cb2, cb2_d[:])
            sc2 = cw.tile([128, 1], F32)
            nc.sync.dma_start(sc2, sc2_d[:])
            sh2 = cw.tile([128, 1], F32)
            nc.sync.dma_start(sh2, sh2_d[:])
            cb3 = cw.tile([128, 2], F32)
            nc.sync.dma_start(cb3, cb3_d[:])
            sc3 = cw.tile([128, 2], F32)
            nc.sync.dma_start(sc3, sc3_d[:])
            sh3 = cw.tile([128, 2], F32)
            nc.sync.dma_start(sh3, sh3_d[:])

            for i in range(8):
                t0 = i * 128
                for dh in range(3):
                    for dw in range(3):
                        s = dh * 3 + dw
                        start = (t0 + dh) * 130 + dw
                        nc.sync.dma_start(x9[s:s + 1, 0:132 * 130],
                                          xp_d[ds(start, 132 * 130)][None, :])
                # ---- conv1: 33 chunks of (4 rows x 128 f)
                for c in range(33):
                    psum = pp1.tile([128, 512], F32, tag="cpsum")
                    rhs = x9[:, c * 520:c * 520 + 520].rearrange(
                        "p (r w) -> p r w", w=130)[:, :, 0:128]
                    nc.tensor.matmul(psum[0:64], w1s, rhs, start=True, stop=True)
                    tmp = tp1.tile([64, 512], BF, tag="c1tmp")
                    nc.scalar.activation(tmp, psum[0:64], AF.Relu, bias=cb1)
                    tr = tmp[:].rearrange("q (r f e) -> q r f e", f=64, e=2)
                    pm = tp1.tile([64, 256], BF, tag="c1pm")
                    pmr = pm[:].rearrange("q (r f) -> q r f", f=64)
                    nc.vector.tensor_tensor(pmr, tr[:, :, :, 0], tr[:, :, :, 1],
                                            OP.max)
                    xv = x2[0:64, c * 264:c * 264 + 264].rearrange(
                        "q (r w) -> q r w", w=66)[:, :, 1:65]
                    nc.vector.scalar_tensor_tensor(
                        xv, pmr, sc1, sh1[:, 0:1, None].to_broadcast(pmr.shape),
                        OP.mult, OP.add)
                if i == 0:
                    nc.vector.memset(x2[0:64, 0:132], 0.0)
                if i == 7:
                    nc.vector.memset(x2[0:64, 130 * 66:132 * 66], 0.0)
                # ---- conv2: 17 chunks of (<=8 rows x 64 f)
                for c in range(17):
                    r0 = c * 8
                    rows = min(8, 130 - r0)
                    nfree = rows * 64
                    psum = pp1.tile([128, 512], F32, tag="cpsum")
                    for si in range(9):
                        dh, dw = si // 3, si % 3
                        off = (r0 + dh) * 66 + dw
                        rhs = x2[:, off:off + rows * 66].rearrange(
                            "p (r w) -> p r w", w=66)[:, :, 0:64]
                        nc.tensor.matmul(psum[:, 0:nfree],
                                         w2s[:, si * 128:(si + 1) * 128],
                                         rhs, start=(si == 0), stop=(si == 8))
                    tmp = tp1.tile([128, 512], BF, tag="c2tmp")
                    nc.scalar.activation(tmp[:, 0:nfree], psum[:, 0:nfree],
                                         AF.Relu, bias=cb2)
                    tr = tmp[:, 0:nfree].rearrange("p (r f e) -> p r f e",
                                                   f=32, e=2)
                    pm = tp1.tile([128, 256], BF, tag="c2pm")
                    pmr = pm[:, 0:rows * 32].rearrange("p (r f) -> p r f", f=32)
                    nc.vector.tensor_tensor(pmr, tr[:, :, :, 0], tr[:, :, :, 1],
                                            OP.max)
                    xv = x3[:, r0 * 34:r0 * 34 + rows * 34].rearrange(
                        "p (r w) -> p r w", w=34)[:, :, 1:33]
                    nc.vector.scalar_tensor_tensor(
                        xv, pmr, sc2, sh2[:, 0:1, None].to_broadcast(pmr.shape),
                        OP.mult, OP.add)
                if i == 0:
                    nc.vector.memset(x3[:, 0:34], 0.0)
                if i == 7:
                    nc.vector.memset(x3[:, 129 * 34:130 * 34], 0.0)
                # ---- conv3: 2 co-chunks x 8 chunks of (16 rows x 32 f)
                for ch in range(2):
                    for c in range(8):
                        r0 = c * 16
                        psum = pp1.tile([128, 512], F32, tag="cpsum")
                        for si in range(9):
                            dh, dw = si // 3, si % 3
                            off = (r0 + dh) * 34 + dw
                            rhs = x3[:, off:off + 16 * 34].rearrange(
                                "p (r w) -> p r w", w=34)[:, :, 0:32]
                            nc.tensor.matmul(
                                psum,
                                w3s[:, (si * 2 + ch) * 128:(si * 2 + ch + 1) * 128],
                                rhs, start=(si == 0), stop=(si == 8))
                        tmp = tp1.tile([128, 512], BF, tag="c3tmp")
                        nc.scalar.activation(tmp, psum, AF.Relu,
                                             bias=cb3[:, ch:ch + 1])
                        tr = tmp[:].rearrange("p (r f e) -> p f r e", f=16, e=2)
                        pm = tp1.tile([128, 256], BF, tag="c3pm")
                        pmr = pm[:].rearrange("p (f r) -> p f r", r=16)
                        nc.vector.tensor_tensor(pmr, tr[:, :, :, 0],
                                                tr[:, :, :, 1], OP.max)
                        yv = ypst[:].rearrange("p (f c t) -> p f c t",
                                               f=16, c=2)[:, :, ch, r0:r0 + 16]
                        nc.vector.scalar_tensor_tensor(
                            yv, pmr, sc3[:, ch:ch + 1],
                            sh3[:, ch:ch + 1, None].to_broadcast(pmr.shape),
                            OP.mult, OP.add)
                nc.sync.dma_start(yp_d[:, :, ds(t0, 128)],
                                  ypst[:].rearrange("p (k t) -> p k t", k=32))

        # ================== phase 2: gx1 ==================
        # full yp resident -> one 32-deep psum accumulation per (d, mc, tch),
        # bias applied on the psum->bf16 store; per-dir pair-AG fired as soon
        # as that direction is staged (fwd AG hides under bwd compute)
        a2t = ar2.tile([128, 2 * 12288], BF, tag="ar2")
        gxst = [a2t[:, 0:12288].rearrange("p (m t) -> p m t", t=1024),
                a2t[:, 12288:].rearrange("p (m t) -> p m t", t=1024)]
        ypsb_t = ar1.tile([128, 32 * 1024], BF, tag="ar1")
        ypsb = ypsb_t[:].rearrange("p (k t) -> p k t", t=1024)
        nc.sync.dma_start(ypsb, yp_d[:])
        with tc.tile_pool(name="wi1sb", bufs=2) as wip, \
             tc.tile_pool(name="p2psum", bufs=4, space="PSUM") as pp2:
            for d in range(2):
                for mc in range(12):
                    wisb = wip.tile([128, 32 * 128], BF, tag="wi1t")
                    nc.sync.dma_start(
                        wisb[:].rearrange("p (k j) -> p k j", k=32),
                        wi1_d[d, mc])
                    for tch in range(2):
                        psum = pp2.tile([128, 512], F32, tag="gxpsum")
                        for kc in range(32):
                            nc.tensor.matmul(
                                psum, wisb[:, kc * 128:(kc + 1) * 128],
                                ypsb[:, kc, ds(tch * 512, 512)],
                                start=(kc == 0), stop=(kc == 31))
                        if d == 1 and NEG_STRIDE_GX:
                            gview = gxst[d][:, mc, 1023 - tch * 512::-1][:, 0:512]
                        else:
                            gview = gxst[d][:, mc, tch * 512:(tch + 1) * 512]
                        nc.vector.tensor_scalar_add(
                            gview, psum, gxb1_sb[d][:, mc:mc + 1])
                nc.sync.dma_start(
                    ag1_in[d],
                    gxst[d].rearrange("p m t -> p (m t)"))
                nc.gpsimd.collective_compute(
                    "AllGather", OP.bypass, replica_groups=PAIR_GROUPS,
                    ins=[ag1_in[d][None, :].opt()],
                    outs=[ag1_out[:].rearrange(
                        "(d s) p f -> d s p f", d=2)[d].opt()])

        # my two chains: dir-major gathered slots {2*sel, 2*sel+1}
        gxs1_t = ar1.tile([128, 12 * 2 * 1024], BF, tag="ar1")
        gxs1 = gxs1_t[:].rearrange("p (m c t) -> p m c t", c=2, t=1024)
        ag1v = ag1_out[:].rearrange("s p f -> p s f")
        for ch in range(2):
            nc.sync.dma_start(gxs1[:, :, ch:ch + 1, :],
                              ag1v[:, ds(2 * sel + ch, 1), :].rearrange(
                                  "p s (m t) -> p m s t", t=1024))

        # ================== phase 3: L1 scans ==================
        ps_ctx = tc.tile_pool(name="scan_ps", bufs=2, space="PSUM")
        ps = ps_ctx.__enter__()
        _scan_loop(nc, tc, vs, ps, nblk1, wh1, gxs1, gxs1[:, 8:12],
                   ident_sb, bhn1l, e8_sb, h1a, h1b, louts1)

        # louts exchange: pairwise AG by chain -> my sample's fwd+bwd
        for ch in range(2):
            nc.sync.dma_start(ag2_in[ch], louts1[:, :, ch, :])
        nc.gpsimd.collective_compute(
            "AllGather", OP.bypass, replica_groups=PAIR_GROUPS,
            ins=[ag2_in[:].opt()], outs=[ag2_out[:].opt()])
        l2in = keep.tile([128, 4 * 512], BF, tag="l2in")
        with tc.tile_pool(name="mid1", bufs=1) as mid1:
            l2pair = mid1.tile([128, 2, 2048], BF, tag="l2pair")
            ag2v = ag2_out[:].rearrange("s p f -> p s f")
            for ch in range(2):
                nc.sync.dma_start(l2pair[:, ch:ch + 1, :],
                                  ag2v[:, ds(sel + 2 * ch, 1), :])
            nc.vector.tensor_tensor(l2in, l2pair[:, 0, :], l2pair[:, 1, :],
                                    OP.add)

        # ================== phase 4: gx2 (own sample, both dirs) ==========
        gx2f_t = ar1.tile([128, 12288], F32, tag="ar1")
        gx2f = gx2f_t[:, 0:6144].rearrange("p (m t) -> p m t", t=512)
        gx2b = gx2f_t[:, 6144:12288].rearrange("p (m t) -> p m t", t=512)
        gx2t = (gx2f, gx2b)
        wi2sb = []
        w4 = ar2.tile([128, 12288], BF, tag="ar2")
        wi2sb = [w4[:, 0:6144], w4[:, 6144:12288]]
        for d in range(2):
            nc.sync.dma_start(wi2sb[d], wi2_d[d])
        with tc.tile_pool(name="p4psum", bufs=2, space="PSUM") as pp4:
            for d in range(2):
                for mc in range(12):
                    psum = pp4.tile([128, 512], F32, tag="gx2psum")
                    for kc in range(4):
                        nc.tensor.matmul(
                            psum,
                            wi2sb[d][:, (mc * 4 + kc) * 128:(mc * 4 + kc + 1) * 128],
                            l2in[:, kc * 512:(kc + 1) * 512],
                            start=(kc == 0), stop=(kc == 3))
                    if d == 1 and NEG_STRIDE_GX:
                        gview = gx2t[d][:, mc, 511::-1][:, 0:512]
                    else:
                        gview = gx2t[d][:, mc, :]
                    nc.vector.tensor_scalar_add(gview, psum,
                                                gxb2_sb[d][:, mc:mc + 1])
        with tc.tile_pool(name="gx2cvt", bufs=2) as gcp2:
            for d in range(2):
                st = gcp2.tile([128, 6144], BF, tag="gx2cvt")
                nc.vector.tensor_copy(st, gx2f_t[:, d * 6144:(d + 1) * 6144])
                nc.sync.dma_start(ag3_in[d], st[:])
        nc.gpsimd.collective_compute(
            "AllGather", OP.bypass, replica_groups=PAIR_GROUPS,
            ins=[ag3_in[:].opt()], outs=[ag3_out[:].opt()])
        gxs2_t = ar2.tile([128, 12 * 2 * 512], BF, tag="ar2")
        gxs2 = gxs2_t[:].rearrange("p (m c t) -> p m c t", c=2, t=512)
        ag3v = ag3_out[:].rearrange("s p f -> p s f")
        for ch in range(2):
            nc.sync.dma_start(gxs2[:, :, ch:ch + 1, :],
                              ag3v[:, ds(sel + 2 * ch, 1), :].rearrange(
                                  "p s (m t) -> p m s t", t=512))

        # ================== phase 5: L2 scans ==================
        _scan_loop(nc, tc, vs, ps, nblk2, wh2, gxs2, gxs2[:, 8:12],
                   ident_sb, bhn2l, e8_sb, h2a, h2b, louts2)

        ps_ctx.__exit__(None, None, None)

        # final: pairwise AG of partial outputs by chain, sum own sample
        for ch in range(2):
            nc.sync.dma_start(ag4_in[ch], louts2[:, :, ch, :])
        nc.gpsimd.collective_compute(
            "AllGather", OP.bypass, replica_groups=PAIR_GROUPS,
            ins=[ag4_in[:].opt()], outs=[ag4_out[:].opt()])
        with tc.tile_pool(name="fin", bufs=1) as fin:
            opair = fin.tile([128, 2, 1024], F32, tag="opair")
            ag4v = ag4_out[:].rearrange("s p f -> p s f")
            for ch in range(2):
                nc.sync.dma_start(opair[:, ch:ch + 1, :],
                                  ag4v[:, ds(sel + 2 * ch, 1), :])
            osb = fin.tile([128, 4 * 256], F32, tag="osb")
            nc.vector.tensor_tensor(osb, opair[:, 0, :], opair[:, 1, :],
                                    OP.add)
            nc.sync.dma_start(out_d[:],
                              osb[:].rearrange("p (k s) -> p k s", k=4))

    nc.compile()
    return nc


# --------------------------------------------------------------------------
# host-side preprocessing
# --------------------------------------------------------------------------

def _bn(g, be, rm, rv):
    s = np.asarray(g) / np.sqrt(np.asarray(rv) + BN_EPS)
    return (s.astype(np.float32),
            (np.asarray(be) - np.asarray(rm) * s).astype(np.float32))


def _prep_common(inputs):
    d = {}
    cw1 = np.asarray(inputs['cw1'])
    w1s = np.zeros((128, 64), np.float32)
    for dh in range(3):
        for dw in range(3):
            w1s[dh * 3 + dw] = cw1[:, 0, dh, dw]
    d['w1s'] = w1s.astype(BF16)
    w2 = np.asarray(inputs['cw2'])
    w2s = np.zeros((9, 128, 128), np.float32)
    w2s[:, 0:64, :] = w2.transpose(2, 3, 1, 0).reshape(9, 64, 128)
    d['w2s'] = w2s.astype(BF16)
    w3 = np.asarray(inputs['cw3'])
    d['w3s'] = np.ascontiguousarray(
        w3.transpose(2, 3, 1, 0).reshape(9, 128, 2, 128)).astype(BF16)
    sc1, sh1 = _bn(inputs['g1'], inputs['be1'], inputs['rm1'], inputs['rv1'])
    sc2, sh2 = _bn(inputs['g2'], inputs['be2'], inputs['rm2'], inputs['rv2'])
    sc3, sh3 = _bn(inputs['g3'], inputs['be3'], inputs['rm3'], inputs['rv3'])
    d['cb1'] = np.asarray(inputs['cb1'], np.float32).reshape(64, 1)
    d['sc1'] = sc1.reshape(64, 1)
    d['sh1'] = sh1.reshape(64, 1)
    d['cb2'] = np.asarray(inputs['cb2'], np.float32).reshape(128, 1)
    d['sc2'] = sc2.reshape(128, 1)
    d['sh2'] = sh2.reshape(128, 1)
    d['cb3'] = np.ascontiguousarray(
        np.asarray(inputs['cb3'], np.float32).reshape(2, 128).T)
    d['sc3'] = np.ascontiguousarray(sc3.reshape(2, 128).T)
    d['sh3'] = np.ascontiguousarray(sh3.reshape(2, 128).T)

    dprime = np.arange(4096)
    perm = (dprime % 256) * 16 + dprime // 256

    wi1 = np.zeros((2, 12, 128, 32, 128), np.float32)
    gxb1 = np.zeros((2, 128, 12), np.float32)
    wh1 = np.zeros((2, 128, 4 * 12 * 128), np.float32)
    bhn1 = np.zeros((2, 128, 4), np.float32)
    wi2 = np.zeros((2, 128, 12 * 4 * 128), np.float32)
    gxb2 = np.zeros((2, 128, 12), np.float32)
    wh2 = np.zeros((2, 128, 4 * 12 * 128), np.float32)
    bhn2 = np.zeros((2, 128, 4), np.float32)
    for di, tag in enumerate('fb'):
        wi = np.asarray(inputs[f'wi{tag}1'])[:, perm]
        wi1[di] = wi.reshape(12, 128, 32, 128).transpose(0, 3, 2, 1)
        bias = np.asarray(inputs[f'bi{tag}1']).copy()
        bh = np.asarray(inputs[f'bh{tag}1'])
        bias[:1024] += bh[:1024]
        gxb1[di] = bias.reshape(12, 128).T
        wh1[di] = np.asarray(inputs[f'wh{tag}1']).reshape(
            12, 128, 4, 128).transpose(3, 2, 0, 1).reshape(128, -1)
        bhn1[di] = bh[1024:].reshape(4, 128).T
        wi2v = np.asarray(inputs[f'wi{tag}2'])
        wi2[di] = wi2v.reshape(12, 128, 4, 128).transpose(
            3, 0, 2, 1).reshape(128, -1)
        bias2 = np.asarray(inputs[f'bi{tag}2']).copy()
        bh2 = np.asarray(inputs[f'bh{tag}2'])
        bias2[:1024] += bh2[:1024]
        gxb2[di] = bias2.reshape(12, 128).T
        wh2[di] = np.asarray(inputs[f'wh{tag}2']).reshape(
            12, 128, 4, 128).transpose(3, 2, 0, 1).reshape(128, -1)
        bhn2[di] = bh2[1024:].reshape(4, 128).T
    d['wi1'] = wi1.astype(BF16)
    d['gxb1'] = gxb1
    d['wi2'] = wi2.astype(BF16)
    d['gxb2'] = gxb2
    d['_wh1'] = wh1.astype(BF16)
    d['_bhn1'] = bhn1
    d['_wh2'] = wh2.astype(BF16)
    d['_bhn2'] = bhn2
    return d


def _bhn_lhst(bhn):
    # lhsT[c, p] = bhn8[p, c] for c < 8 (out[p, c] = lhsT[c, p] under rhs=e8)
    bhn8 = np.repeat(bhn, 2, axis=1)          # [128, 8] (j c)
    lhst = np.zeros((128, 128), np.float32)
    lhst[0:8, :] = bhn8.T
    return lhst.astype(BF16)


def _prep_sample(x_c):
    xp = np.zeros((1031, 130), np.float32)
    xp[3:1027, 1:129] = x_c
    return {'xp': xp.astype(BF16).reshape(-1)}


def get_nc(nblk1=16, nblk2=8):
    key = (nblk1, nblk2)
    if key not in _CACHED_NC:
        _CACHED_NC[key] = build_nc(nblk1, nblk2)
    return _CACHED_NC[key]


def run(inputs, nblk1=16, nblk2=8, **kwargs):
    nc = get_nc(nblk1, nblk2)
    common = _prep_common(inputs)
    private = {k: common.pop(k) for k in list(common) if k.startswith('_')}
    x = np.asarray(inputs['x'])
    in_maps = []
    for c in range(8):
        m = dict(common)
        m.update(_prep_sample(x[c, 0]))
        par = c % 2
        m['wh1m'] = private['_wh1'][par]
        m['bhn1m'] = _bhn_lhst(private['_bhn1'][par])
        m['wh2m'] = private['_wh2'][par]
        m['bhn2m'] = _bhn_lhst(private['_bhn2'][par])
        m['ident'] = np.eye(128, dtype=BF16)
        m['e8'] = np.eye(128, dtype=BF16)[:, :8].copy()
        m['dsel'] = np.array([[par]], dtype=np.uint32)
        in_maps.append(m)
    return run_bass_kernel_spmd(nc, in_maps, core_ids=list(range(8)), **kwargs)


def kernel(**inputs) -> np.ndarray:
    res = run(inputs)
    outs = []
    for c in range(8):
        o = np.asarray(res.results[c]['out'])  # [128, 4, 256]
        outs.append(np.ascontiguousarray(
            o.transpose(2, 1, 0).reshape(256, 512)))
    return np.stack(outs).astype(np.float32)
